# revision 40
# baseline (speedup 1.0000x reference)
"""Multi-head attention (S=2048, B=2, D=1024, H=16) on 8 Trainium2 cores.

Sharding: tensor-parallel over heads (4 groups of 4 heads) x data-parallel
over batch (2). Core r handles batch r//4, heads [4*(r%4), 4*(r%4)+4).
Each core projects its 256 channels and runs attention for its 4 heads.
The per-core ctx slices ([256 ch, 512 tok] per tq block) are AllGathered
across each 4-core batch group (4x less wire than ReduceScattering output
partials), then each core computes the output projection for all tokens of
the block but only its own 256 output-channel columns of Wo (the e-slice
lives in the per-core wo_t input, keeping the program SPMD-uniform with no
redundant compute); the host concatenates the column slices.

All tensors are bf16 on the wire and in SBUF (fp32 accumulation in PSUM).
Softmax denominators come free from an extra ones-column appended to V in
the PV matmul. V's bias and the output bias are folded out algebraically
and added on the host. The exp ACTs own the Scalar queue exclusively during
attention; the attention phase is paced by ScalarE exp throughput, with the
Q projection, V projection (block 0) and output projections tucked into the
PE's idle cycles under the exp shadow.
"""
import sys

sys.path.insert(0, "/opt/trn_rl_repo")

import numpy as np
import concourse.bacc as bacc
import concourse.mybir as mybir
from concourse import tile
from concourse.bass_utils import run_bass_kernel_spmd

dt = mybir.dt
AF = mybir.ActivationFunctionType
BF16 = np.dtype(mybir.dt.np(dt.bfloat16))

S, B, D = 2048, 2, 1024
H, DK = 16, 64
NCORES = 8
HC = 4                 # heads per core
CH = HC * DK           # 256 local channels per core
SCALE = np.float32(1.0 / np.sqrt(DK))
GROUPS = [[0, 1, 2, 3], [4, 5, 6, 7]]

TQ = 512               # tq block (matmul free dim)
NTQ = S // TQ          # 4
NKD = D // 128         # 8 contraction tiles for projections
NTK = S // 128         # 16 key tiles


def build_nc():
    f32, bf = dt.float32, dt.bfloat16
    nc = bacc.Bacc("TRN2", target_bir_lowering=False, debug=False,
                   num_devices=NCORES)

    xq = nc.dram_tensor("xq_t", [D, S], bf, kind="ExternalInput").ap()
    xk = nc.dram_tensor("xk_t", [D, S], bf, kind="ExternalInput").ap()
    xv = nc.dram_tensor("xv_t", [D, S], bf, kind="ExternalInput").ap()
    wq = nc.dram_tensor("wq_t", [D, CH], bf, kind="ExternalInput").ap()
    wk = nc.dram_tensor("wk_t", [D, CH], bf, kind="ExternalInput").ap()
    wv = nc.dram_tensor("wv_t", [D, CH], bf, kind="ExternalInput").ap()
    # Wo e-column slice: core r owns output channels [256*(r%4), +256)
    wo = nc.dram_tensor("wo_t", [D, CH], bf, kind="ExternalInput").ap()
    bq = nc.dram_tensor("bq", [2, 128], f32, kind="ExternalInput").ap()
    bk = nc.dram_tensor("bk", [2, 128], f32, kind="ExternalInput").ap()
    out_ext = nc.dram_tensor("out_esl", [S, CH], bf,
                             kind="ExternalOutput").ap()

    with tile.TileContext(nc) as tc:
        with tc.tile_pool(name="const", bufs=1) as cp, \
             tc.tile_pool(name="stream", bufs=1) as sp, \
             tc.tile_pool(name="psum", bufs=1, space="PSUM") as pp, \
             tc.tile_pool(name="dram", bufs=1, space="DRAM") as dp:

            # ---- resident weights / inputs (few large DMAs) ----
            wq_sb = cp.tile([128, NKD, CH], bf, tag="wq", name="wq_sb")
            wk_sb = cp.tile([128, NKD, CH], bf, tag="wk", name="wk_sb")
            wv_sb = cp.tile([128, NKD, CH], bf, tag="wv", name="wv_sb")
            wo_sb = cp.tile([128, NKD, CH], bf, tag="wo", name="wo_sb")
            bq_sb = [cp.tile([128, 1], f32, tag=f"bq{j}", name=f"bq{j}")
                     for j in range(2)]
            bk_sb = [cp.tile([128, 1], f32, tag=f"bk{j}", name=f"bk{j}")
                     for j in range(2)]
            xk_sb = cp.tile([128, NKD, S], bf, tag="xk", name="xk_sb")

            wk_v = wk.rearrange("(k p) c -> p k c", p=128)
            wq_v = wq.rearrange("(k p) c -> p k c", p=128)
            wv_v = wv.rearrange("(k p) c -> p k c", p=128)
            wo_v = wo.rearrange("(k p) c -> p k c", p=128)
            xk_v = xk.rearrange("(k p) s -> p k s", p=128)
            xq_v = xq.rearrange("(k p) s -> p k s", p=128)
            xv_v = xv.rearrange("(k p) s -> p k s", p=128)

            # xq/xv stream through double-buffered chunk tiles
            xq_tiles, xv_tiles = {}, {}

            def fetch_x(tiles, dram_v, t, tag, eng):
                xt = sp.tile([128, NKD, TQ], bf, tag=tag, bufs=2,
                             name=f"{tag}{t}")
                eng.dma_start(xt[:], dram_v[:, :, t * TQ:(t + 1) * TQ])
                tiles[t] = xt

            # scalar queue: weights, biases, xv; sync queue: xk then xq
            nc.scalar.dma_start(wk_sb[:], wk_v)
            nc.scalar.dma_start(wq_sb[:], wq_v)
            for j in range(2):
                nc.scalar.dma_start(bq_sb[j][:], bq[j].unsqueeze(1))
                nc.scalar.dma_start(bk_sb[j][:], bk[j].unsqueeze(1))
            nc.scalar.dma_start(wv_sb[:], wv_v)
            fetch_x(xv_tiles, xv_v, 0, "xv", nc.scalar)
            fetch_x(xv_tiles, xv_v, 1, "xv", nc.scalar)
            nc.scalar.dma_start(wo_sb[:], wo_v)
            # first half-chunk split so the very first K-proj matmuls are
            # gated on 1 MB of DMA instead of 2
            nc.sync.dma_start(xk_sb[:, :, 0:256], xk_v[:, :, 0:256])
            nc.sync.dma_start(xk_sb[:, :, 256:512], xk_v[:, :, 256:512])
            for t in range(1, 4):
                cs = slice(t * TQ, (t + 1) * TQ)
                nc.sync.dma_start(xk_sb[:, :, cs], xk_v[:, :, cs])
            fetch_x(xq_tiles, xq_v, 0, "xq", nc.sync)

            # ---- persistent activations ----
            qc = [cp.tile([128, S], bf, tag=f"qc{j}", name=f"qc{j}")
                  for j in range(2)]
            kc = [cp.tile([128, S], bf, tag=f"kc{j}", name=f"kc{j}")
                  for j in range(2)]
            # V tiles: [token128, 4*(64 V + 1 ones)] per key tile
            vt = [cp.tile([128, HC * (DK + 1)], bf, tag=f"vt{t}",
                          name=f"vt{t}") for t in range(NTK)]
            ctx = [cp.tile([128, S], bf, tag=f"ctx{j}", name=f"ctx{j}")
                   for j in range(2)]
            for t in range(NTK):
                vt_view = vt[t][:].rearrange("p (h c) -> p h c", h=HC)
                nc.vector.memset(vt_view[:, :, DK:DK + 1], 1.0)
            # [1, 64] ones row: stationary of the 1/den broadcast
            # outer-product (f32r so the PE runs it at full rate)
            ones64f = cp.tile([1, DK], f32, tag="ones64f", name="ones64f")
            nc.vector.memset(ones64f[:], 1.0)
            ones64 = cp.tile([1, DK], dt.float32r, tag="ones64",
                             name="ones64")
            nc.vector.tensor_copy(ones64[:], ones64f[:])

            # warm the exp table while the prologue DMAs run
            warm = sp.tile([1, 1], f32, tag="warm", name="warm")
            nc.vector.memset(warm[:], 0.0)
            nc.scalar.activation(warm[:], warm[:], AF.Exp)
            # warm the collective stream so the first real AllGather is fast
            wci = dp.tile([128, 16], bf, tag="wci", name="warm_cc_in")
            wco = dp.tile([512, 16], bf, tag="wco", name="warm_cc_out")
            wcs = sp.tile([128, 16], bf, tag="wcs", name="warm_cc_sb")
            nc.vector.memset(wcs[:], 0.0)
            nc.sync.dma_start(wci[:], wcs[:])
            nc.gpsimd.collective_compute(
                "AllGather", mybir.AluOpType.bypass, replica_groups=GROUPS,
                ins=[wci[:]], outs=[wco[:]])

            def proj_qk(x_view, w_sb, b_sb, dst, t, j, c0=0, c1=TQ):
                """One token x 128-channel chunk of a Q/K projection.

                x_view: [128, NKD, TQ] SBUF view of tokens [t*TQ, (t+1)*TQ);
                c0:c1 selects a column sub-range of the chunk.
                """
                cs = slice(t * TQ + c0, t * TQ + c1)
                ps = pp.tile([128, c1 - c0], f32, tag="po", bufs=2,
                             name=f"ps{t}_{j}_{c0}")
                for k in range(NKD):
                    nc.tensor.matmul(
                        ps[:], w_sb[:, k, j * 128:(j + 1) * 128],
                        x_view[:, k, c0:c1],
                        start=(k == 0), stop=(k == NKD - 1))
                nc.vector.tensor_scalar_add(dst[j][:, cs], ps[:],
                                            b_sb[j][:, 0:1])

            def proj_v_chunk(tt):
                """Four key-tiles of the V projection (tokens 512tt..+512)."""
                xvt = xv_tiles[tt]
                for ts in range(4):
                    t = tt * 4 + ts
                    pv = pp.tile([128, CH], f32, tag="po", bufs=2,
                                 name=f"pv{t}")
                    for k in range(NKD):
                        nc.tensor.matmul(
                            pv[:],
                            xvt[:, k, ts * 128:(ts + 1) * 128],
                            wv_sb[:, k, :],
                            start=(k == 0), stop=(k == NKD - 1))
                    dst_view = vt[t][:].rearrange("p (h c) -> p h c", h=HC)
                    src_view = pv[:].rearrange("p (h c) -> p h c", h=HC)
                    nc.vector.tensor_copy(dst_view[:, :, 0:DK], src_view)

            # ---- prologue: K proj, Q proj (block 0), V chunk 0 ----
            for t in range(4):
                for j in range(2):
                    if t == 0 and j == 0:
                        proj_qk(xk_sb[:, :, 0:TQ], wk_sb, bk_sb, kc,
                                0, 0, 0, 256)
                        proj_qk(xk_sb[:, :, 0:TQ], wk_sb, bk_sb, kc,
                                0, 0, 256, TQ)
                    else:
                        proj_qk(xk_sb[:, :, t * TQ:(t + 1) * TQ],
                                wk_sb, bk_sb, kc, t, j)
            for j in range(2):
                proj_qk(xq_tiles[0][:], wq_sb, bq_sb, qc, 0, j)
            proj_v_chunk(0)

            # ---- AllGather buffers (per tq block) ----
            cc_ins = [dp.tile([2 * 128, TQ], bf, tag=f"ccin{b}",
                              name=f"cc_in{b}") for b in range(NTQ)]
            cc_outs = [dp.tile([8 * 128, TQ], bf, tag=f"ccout{b}",
                               name=f"cc_out{b}") for b in range(NTQ)]

            def emit_outproj_tchunk(b, tc_, ctxg):
                """Out-projection (my 256 e-columns) for the 128 tokens
                [b*512+tc_*128, +128)."""
                t0 = tc_ * 128
                osb = sp.tile([128, CH], bf, tag="ot", bufs=3,
                              name=f"ot{b}_{tc_}")
                po = pp.tile([128, CH], f32, tag="po", bufs=2,
                             name=f"po{b}_{tc_}")
                for k in range(NKD):
                    nc.tensor.matmul(
                        po[:],
                        ctxg[:, k, t0:t0 + 128],
                        wo_sb[:, k, :],
                        start=(k == 0), stop=(k == NKD - 1))
                nc.vector.tensor_copy(osb[:], po[:])
                nc.sync.dma_start(
                    out_ext[b * TQ + t0:b * TQ + t0 + 128, :], osb[:])

            ctxg_tiles = {}

            def fetch_ctxg(b):
                """DMA block b's gathered ctx (after its AllGather lands)."""
                ctxg = sp.tile([128, NKD, TQ], bf, tag="ctxg",
                               bufs=2, name=f"ctxg{b}")
                nc.sync.dma_start(
                    ctxg[:],
                    cc_outs[b][:].rearrange("(k p) s -> p k s", p=128))
                ctxg_tiles[b] = ctxg

            def norm_front(tqi, p, cx):
                """DVE part of softmax normalize: evacuate cx, 1/den."""
                outs = []
                for h in range(2):
                    cxs = sp.tile([65, TQ], f32, tag="cxs", bufs=4,
                                  name=f"cxs{p}_{h}")
                    nc.vector.tensor_copy(cxs[:], cx[h][:])
                    den = sp.tile([1, TQ], f32, tag="den", bufs=4,
                                  name=f"den{p}_{h}")
                    nc.vector.tensor_copy(den[:], cxs[64:65, :])
                    rc = sp.tile([1, TQ], f32, tag="rc", bufs=4,
                                 name=f"rc{p}_{h}")
                    nc.vector.reciprocal_approx_fast(rc[:], den[:])
                    rcr = sp.tile([1, TQ], dt.float32r, tag="rcr",
                                  bufs=4, name=f"rcr{p}_{h}")
                    nc.vector.tensor_copy(rcr[:], rc[:])
                    outs.append((cxs, rcr))
                return outs

            def norm_back(tqi, p, fr):
                """PE outer-product 1/den broadcast + multiply into ctx;
                for p==1 also ship the block's ctx and AllGather it."""
                tq0 = tqi * TQ
                for h in range(2):
                    cxs, rcr = fr[h]
                    bc = pp.tile([64, TQ], f32, tag="po", bufs=2,
                                 name=f"bc{p}_{h}")
                    nc.tensor.matmul(bc[:], ones64[:], rcr[:],
                                     start=True, stop=True)
                    nc.vector.tensor_mul(
                        ctx[p][h * 64:(h + 1) * 64, tq0:tq0 + TQ],
                        cxs[0:64, :], bc[:])
                if p == 1:
                    for j in range(2):
                        nc.sync.dma_start(
                            cc_ins[tqi][j * 128:(j + 1) * 128, :],
                            ctx[j][:, tq0:tq0 + TQ])
                    nc.gpsimd.collective_compute(
                        "AllGather", mybir.AluOpType.bypass,
                        replica_groups=GROUPS,
                        ins=[cc_ins[tqi][:]], outs=[cc_outs[tqi][:]])

            pending_norm = None

            for tqi in range(NTQ):
                tq0 = tqi * TQ
                for p in range(2):             # head pairs (2p, 2p+1)
                    cx = [pp.tile([65, TQ], f32, tag="cx", bufs=2,
                                  name=f"cx{p}_{h}") for h in range(2)]
                    if p == 0 and tqi < NTQ - 1:
                        # next block's Q input, ahead of the blocking
                        # ctxg fetch in the sync queue
                        fetch_x(xq_tiles, xq_v, tqi + 1, "xq", nc.sync)
                    for tk in range(NTK):
                        if tk == 1 and pending_norm is not None:
                            # previous pair's normalize back-half: the PE
                            # outer-products run here, after the DVE chain
                            # has certainly produced 1/den (no PE wait)
                            norm_back(*pending_norm)
                            pending_norm = None
                            # the fetch must be emitted after the AllGather
                            # it depends on (program order = dep order)
                            if p == 0 and tqi > 0:
                                fetch_ctxg(tqi - 1)
                        if tqi == 0 and p == 0 and tk in (0, 4):
                            fetch_x(xv_tiles, xv_v, 2 + tk // 4, "xv",
                                    nc.sync)
                        # out-projection t-chunks, balanced across the two
                        # pairs so neither exceeds the exp-ACT shadow
                        if p == 1 and tqi >= 1 and tk in (8, 12):
                            emit_outproj_tchunk(tqi - 1, (tk - 8) // 4,
                                                ctxg_tiles[tqi - 1])
                        if p == 0 and tqi >= 2 and tk in (4, 10):
                            emit_outproj_tchunk(tqi - 2, 2 + (tk - 4) // 6,
                                                ctxg_tiles[tqi - 2])
                        if tqi == 0 and p == 0 and tk in (2, 6, 10):
                            proj_v_chunk(tk // 4 + 1)
                        # next block's Q projection: one 128-channel half
                        # per pair (j=0 feeds the next p0, j=1 the next p1)
                        if tqi < NTQ - 1 and tk == 13:
                            proj_qk(xq_tiles[tqi + 1][:], wq_sb, bq_sb, qc,
                                    tqi + 1, p)
                        # both heads' scores side by side in one 2-bank tile
                        s1 = pp.tile([128, 2 * TQ], f32, tag="s1", bufs=2,
                                     name=f"s1{tk}")
                        et = sp.tile([128, 2 * TQ], bf, tag="et", bufs=4,
                                     name=f"et{tk}")
                        for h in range(2):      # adjacent -> row-pack overlap
                            r0 = h * 64
                            nc.tensor.matmul(
                                s1[:, h * TQ:(h + 1) * TQ],
                                kc[p][r0:r0 + 64, tk * 128:(tk + 1) * 128],
                                qc[p][r0:r0 + 64, tq0:tq0 + TQ],
                                start=True, stop=True)
                        # one ACT instruction covers both heads
                        nc.scalar.activation(et[:], s1[:], AF.Exp)
                        for h in range(2):
                            hl = p * 2 + h
                            nc.tensor.matmul(
                                cx[h][:],
                                vt[tk][:, hl * 65:(hl + 1) * 65],
                                et[:, h * TQ:(h + 1) * TQ],
                                start=(tk == 0), stop=(tk == NTK - 1))
                    pending_norm = (tqi, p, norm_front(tqi, p, cx))

            # final pair's normalize, then the remaining out-projections
            # (block NTQ-2's last chunks keep the PE warm under the last AG)
            norm_back(*pending_norm)
            for tc_ in (2, 3):
                emit_outproj_tchunk(NTQ - 2, tc_, ctxg_tiles[NTQ - 2])
            fetch_ctxg(NTQ - 1)
            for tc_ in range(4):
                emit_outproj_tchunk(NTQ - 1, tc_, ctxg_tiles[NTQ - 1])

    nc.finalize()
    return nc


_NC = None


def _get_nc():
    global _NC
    if _NC is None:
        _NC = build_nc()
    return _NC


def make_in_maps(q, k, v, Wq, bq, Wk, bk, Wv, bv, Wo, bo):
    """Shard + precondition full inputs into per-core input maps."""
    xq_b = [np.ascontiguousarray(q[:, b, :].T).astype(BF16) for b in range(B)]
    xk_b = [np.ascontiguousarray(k[:, b, :].T).astype(BF16) for b in range(B)]
    xv_b = [np.ascontiguousarray(v[:, b, :].T).astype(BF16) for b in range(B)]
    in_maps = []
    for r in range(NCORES):
        b = r // 4
        g = r % 4
        ch = slice(g * CH, (g + 1) * CH)
        in_maps.append({
            "xq_t": xq_b[b], "xk_t": xk_b[b], "xv_t": xv_b[b],
            "wq_t": np.ascontiguousarray((Wq[ch, :] * SCALE).T).astype(BF16),
            "wk_t": np.ascontiguousarray(Wk[ch, :].T).astype(BF16),
            "wv_t": np.ascontiguousarray(Wv[ch, :].T).astype(BF16),
            # e-column slice of the output projection (torch Linear: Wo[e,d])
            "wo_t": np.ascontiguousarray(Wo[ch, :].T).astype(BF16),
            "bq": (bq[ch] * SCALE).reshape(2, 128).astype(np.float32),
            "bk": bk[ch].reshape(2, 128).astype(np.float32),
        })
    return in_maps


def assemble(results, Wo, bv, bo):
    """Concatenate each core's 256 output-channel columns."""
    out = np.empty((S, B, D), dtype=np.float32)
    for r in range(NCORES):
        b, g = r // 4, r % 4
        out[:, b, g * CH:(g + 1) * CH] = np.asarray(
            results[r]["out_esl"]).astype(np.float32)
    out += (bo + Wo @ bv).astype(np.float32)
    return out


def run_sharded(inputs, trace=False):
    nc = _get_nc()
    in_maps = make_in_maps(**inputs)
    res = run_bass_kernel_spmd(nc, in_maps, list(range(NCORES)), trace=trace)
    full = assemble(res.results, np.asarray(inputs["Wo"], dtype=np.float32),
                    np.asarray(inputs["bv"], dtype=np.float32),
                    np.asarray(inputs["bo"], dtype=np.float32))
    return full, res


def kernel(**inputs) -> np.ndarray:
    inputs = {k_: np.asarray(v_, dtype=np.float32)
              for k_, v_ in inputs.items()}
    full, _ = run_sharded(inputs)
    return full
